# revision 27
# baseline (speedup 1.0000x reference)
"""AttnBlock (GroupNorm + single-head self-attention + residual) on 8 Trainium2 cores.

Sharding: core i handles batch b = i//2 and query-half h = i%2 (2048 of 4096
pixels). Each core computes full-batch groupnorm stats + K/V^T, its half of Q,
attention over all 4096 keys for its 2048 queries, and the output projection.
Host does the final bias + residual add and gathers.

v2 schedule notes (vs v1):
  - x is rolled per core so this core's query half is always columns 0:2048 of
    xkv; Q projects from a slice of xkv (no separate xq DMA).
  - x DMA is chunked (4x 512KB); groupnorm stats pipeline per chunk.
  - rstd = exp(-0.5*ln(var+eps)) so every ACT func (Square/Ln/Exp/Copy/
    Identity) lives in one table set -> single table load.
  - C**-0.25 folded into the q/k weight scaling; K/Q evacuations are pure
    bias-adds on ScalarE, V evacuations on VectorE (splits the PSUM-
    evacuation load across both engines; v1 put everything on VectorE).
  - 1/OSC folded into wot on the host; o8/ot evacuations run on VectorE
    (idle during attention), final-tile ot evacuations on ScalarE.
  - PE warm-up matmuls read stats scratch / weight tiles (dependency-gated)
    instead of dedicated memset tiles.

All matmuls fp8 DoubleRow (fp32 PSUM accumulation); softmax stats in fp32.
Layouts (no on-chip transpose anywhere):
  - Q, K as [c, pixel]   (projection natural layout)
  - V^T as [pixel, c]    (computed directly: lhsT = x blocks)
  - scores S^T[k, q]     (lhsT = K blocks, rhs = Q)
  - softmax sum over k (partition dim) via ones-vector matmul
  - attention out o[c, q](lhsT = V^T blocks, rhs = E^T)
  - o-proj out[q, c']    (lhsT = o blocks, rhs = wo^T) -> per-partition 1/Z scale
"""

import numpy as np
import ml_dtypes

C = 512
HW = 4096
HWQ = 2048
CCH = 4          # channel chunks of 128
KT = 32          # key tiles of 128
QT = 4           # query tiles of 512
NCORES = 8
GS = 16          # channels per group
EPS = 1e-5
SCALE = 1.0 / float(np.sqrt(C))
SCALE_H = float(SCALE ** 0.5)
OSC = 1.0 / 32.0   # o scaled into fp8 range; 1/OSC is folded into wot on host

_cache = {}


def _emit_body(nc, tc, bassmod, mybir, ctx, T):
    """Emit one full forward pass. T is the dict of dram tensor handles."""
    bass = bassmod
    f32 = mybir.dt.float32
    f8 = mybir.dt.float8e4
    f16 = mybir.dt.float16
    AF = mybir.ActivationFunctionType
    ALU = mybir.AluOpType
    DR = mybir.MatmulPerfMode.DoubleRow

    # ---------------- pools ----------------
    consts = ctx.enter_context(tc.tile_pool(name="consts", bufs=1))
    xb = ctx.enter_context(tc.tile_pool(name="xb", bufs=1))
    ps_s = ctx.enter_context(tc.tile_pool(name="ps_s", bufs=3, space="PSUM"))
    ps_o = ctx.enter_context(tc.tile_pool(name="ps_o", bufs=4, space="PSUM"))
    ps_z = ctx.enter_context(tc.tile_pool(name="ps_z", bufs=1, space="PSUM"))
    kpool = ctx.enter_context(tc.tile_pool(name="kpool", bufs=1))
    qpool = ctx.enter_context(tc.tile_pool(name="qpool", bufs=1))
    vpool = ctx.enter_context(tc.tile_pool(name="vpool", bufs=KT // 2))
    opool = ctx.enter_context(tc.tile_pool(name="opool", bufs=1))
    epool = ctx.enter_context(tc.tile_pool(name="epool", bufs=4))
    outp = ctx.enter_context(tc.tile_pool(name="outp", bufs=3))
    rzp = ctx.enter_context(tc.tile_pool(name="rzp", bufs=2))
    spool = ctx.enter_context(tc.tile_pool(name="spool", bufs=1))
    tmpp = ctx.enter_context(tc.tile_pool(name="tmpp", bufs=2))

    # ---------------- input DMAs ----------------
    # Descriptor generation is ~650ns per dma_start and serial on the sync
    # queue: x chunks go FIRST (stats gate on them), consts after.
    xkv_v = xb.tile([128, CCH, HW], f8, tag="xkv", name="xkv")
    for ci in range(CCH):
        nc.sync.dma_start(out=xkv_v[:, ci, :],
                          in_=bass.AP(T["xkv"], ci * 128 * HW, [[HW, 128], [1, HW]]))
    # colc (16 cols) + gadj (128 cols) merged into one [128, 144] input
    cg_sb = consts.tile([128, 144], f32, tag="cg", name="cg")
    nc.sync.dma_start(out=cg_sb, in_=T["cg"][:, :])
    cc_sb = cg_sb[:, 0:16]
    gadj_sb = cg_sb[:, 16:144]
    bv_row = consts.tile([1, C], f32, tag="bvrow", name="bvrow")
    nc.sync.dma_start(out=bv_row, in_=bass.AP(T["bvr"], 0, [[0, 1], [1, C]]))
    gnw_c = [cc_sb[:, 4 * ci + 0:4 * ci + 1] for ci in range(CCH)]
    gnb_c = [cc_sb[:, 4 * ci + 1:4 * ci + 2] for ci in range(CCH)]
    bq_c = [cc_sb[:, 4 * ci + 2:4 * ci + 3] for ci in range(CCH)]
    bk_c = [cc_sb[:, 4 * ci + 3:4 * ci + 4] for ci in range(CCH)]
    # fp8 weights for q/k/v: [128, nm, ci, co]; fp8 wot (pre-scaled by 1/OSC)
    wall = consts.tile([128, 3 * CCH * 512], f8, tag="wall", name="wall")
    nc.sync.dma_start(out=wall, in_=T["wall"][:, :])
    wv_q = {nm: wall[:, i * 2048:(i + 1) * 2048].rearrange("p (c w) -> p c w", c=CCH)
            for i, nm in enumerate(("wkt", "wvt", "wqt"))}
    wot = consts.tile([128, CCH, 512], f8, tag="wot", name="wot")
    nc.sync.dma_start(out=wot, in_=T["wotp"][:, :].rearrange("p (c w) -> p c w", c=CCH))
    ones2_sb = consts.tile([128, 2, 16], f8, tag="ones2", name="ones2")
    nc.vector.memset(ones2_sb, 1.0)
    ones1f = consts.tile([1, 128], f32, tag="ones1f", name="ones1f")
    nc.vector.memset(ones1f, 1.0)
    eps_sb = consts.tile([128, 1], f32, tag="eps", name="eps")
    nc.vector.memset(eps_sb, EPS)

    # PE warm-up: HAM needs ~3.4us of activity to unthrottle and re-throttles
    # after ~3.4us idle. Warm matmuls read the stats scratch of each chunk so
    # they spread across the stats prefix via data deps (no dedicated gates).
    ps_w = ps_z.tile([1, 512], f32, tag="z", name="z")
    _warm = {"first": True}

    def warm_round(rhs3, n=6, last=False):
        for i in range(n):
            nc.tensor.matmul(out=ps_w, lhsT=ones2_sb[:, :, 0:1], rhs=rhs3,
                             perf_mode=DR, start=_warm["first"],
                             stop=(last and i == n - 1), skip_group_check=True)
            _warm["first"] = False

    # ---------------- groupnorm stats (pipelined per chunk) ----------------
    # Per chunk: plain sum on VectorE (chunk 0: GpSimd/Pool, the only engine
    # otherwise idle in the prefix), sum-of-squares via ScalarE Square+accum.
    # The last chunk's stats gate the weight scaling, so spreading the passes
    # across three engines shortens the serial prefix.
    a_pc, aqk_pc = [], []
    b8 = spool.tile([128, CCH], f8, tag="b8", name="b8")
    var4 = spool.tile([128, CCH], f32, tag="var4", name="var4")
    mu4 = spool.tile([128, CCH], f32, tag="mu4", name="mu4")
    for ci in range(CCH):
        st = spool.tile([128, 2], f32, tag=f"st{ci}", name=f"st{ci}")
        nc.vector.reduce_sum(out=st[:, 0:1], in_=xkv_v[:, ci, :],
                             axis=mybir.AxisListType.X)
        scr = tmpp.tile([128, HW], f8, tag="scr", name="scr")
        nc.scalar.activation(out=scr, in_=xkv_v[:, ci, :], func=AF.Square,
                             accum_out=st[:, 1:2])
        ps_g = ps_s.tile([128, 2], f32, tag="ps", name="ps")
        nc.tensor.matmul(out=ps_g, lhsT=gadj_sb, rhs=st, start=True, stop=True)
        gs = spool.tile([128, 2], f32, tag=f"gs{ci}", name=f"gs{ci}")
        nc.scalar.mul(out=gs, in_=ps_g, mul=1.0 / (GS * HW))   # [mu | E[x^2]]
        nc.vector.tensor_copy(mu4[:, ci:ci + 1], gs[:, 0:1])
        var = var4[:, ci:ci + 1]
        nc.vector.tensor_mul(var, gs[:, 0:1], gs[:, 0:1])
        nc.vector.tensor_sub(var, gs[:, 1:2], var)
        warm_round(scr[:, 0:1024].rearrange("p (a b) -> p a b", a=2))
    # rstd = (var+eps)^-0.5 = exp(-0.5*ln(var+eps)), batched over chunks:
    # Ln/Exp live in the resident table set, so no mid-kernel table reloads.
    lnv = spool.tile([128, CCH], f32, tag="lnv", name="lnv")
    nc.scalar.activation(out=lnv, in_=var4, func=AF.Ln, bias=eps_sb, scale=1.0)
    rstd4 = spool.tile([128, CCH], f32, tag="rstd4", name="rstd4")
    nc.scalar.activation(out=rstd4, in_=lnv, func=AF.Exp, scale=-0.5)
    for ci in range(CCH):
        a = spool.tile([128, 1], f32, tag=f"apc{ci}", name=f"apc{ci}")
        nc.vector.tensor_mul(a, rstd4[:, ci:ci + 1], gnw_c[ci])
        aqk = spool.tile([128, 1], f32, tag=f"aqk{ci}", name=f"aqk{ci}")
        nc.vector.tensor_scalar_mul(aqk, a, SCALE_H)
        b = spool.tile([128, 1], f32, tag=f"bpc{ci}", name=f"bpc{ci}")
        nc.vector.tensor_mul(b, mu4[:, ci:ci + 1], a)
        nc.vector.tensor_sub(b, gnb_c[ci], b)
        nc.vector.tensor_copy(b8[:, ci:ci + 1], b)
        a_pc.append(a)
        aqk_pc.append(aqk)

    # effective projection biases: beta = W @ b + bias  (original weights, fp8 matvecs)
    bqeff, bkeff = [], []
    for co in range(CCH):
        psq = ps_s.tile([128, 1], f32, tag="ps", name="ps")
        psk = ps_s.tile([128, 1], f32, tag="ps", name="ps")
        for ci in range(CCH):
            nc.tensor.matmul(out=psk, lhsT=wv_q["wkt"][:, ci, co * 128:(co + 1) * 128],
                             rhs=b8[:, ci:ci + 1], start=(ci == 0), stop=(ci == CCH - 1))
        for ci in range(CCH):
            nc.tensor.matmul(out=psq, lhsT=wv_q["wqt"][:, ci, co * 128:(co + 1) * 128],
                             rhs=b8[:, ci:ci + 1], start=(ci == 0), stop=(ci == CCH - 1))
        bk = spool.tile([128, 1], f32, tag=f"bke{co}", name=f"bke{co}")
        nc.scalar.activation(out=bk, in_=psk, func=AF.Identity, bias=bk_c[co], scale=SCALE_H)
        bq = spool.tile([128, 1], f32, tag=f"bqe{co}", name=f"bqe{co}")
        nc.scalar.activation(out=bq, in_=psq, func=AF.Identity, bias=bq_c[co], scale=SCALE_H)
        bqeff.append(bq)
        bkeff.append(bk)
    psv = ps_s.tile([1, C], f32, tag="ps", name="ps")
    for ci in range(CCH):
        nc.tensor.matmul(out=psv, lhsT=b8[:, ci:ci + 1], rhs=wv_q["wvt"][:, ci, :],
                         start=(ci == 0), stop=(ci == CCH - 1))
    bve_row = spool.tile([1, C], f32, tag="bverow", name="bverow")
    nc.vector.tensor_add(bve_row, psv, bv_row)
    # broadcast bve across partitions via outer-product matmul (no DRAM trip)
    ps_bvb = ps_s.tile([128, C], f32, tag="ps", name="ps")
    nc.tensor.matmul(out=ps_bvb, lhsT=ones1f, rhs=bve_row, start=True, stop=True)
    bvb_sb = consts.tile([128, C], f32, tag="bvb", name="bvb")
    nc.vector.tensor_copy(bvb_sb, ps_bvb)

    # scale q/k weight rows by a*C**-0.25 and v rows by a (in place, after the
    # beta matvecs read them); alternate DVE/ACT to split the chain
    warm_round(wv_q["wvt"][:, 0:2, 0:512], last=True)
    for nm, avec in (("wkt", aqk_pc), ("wvt", a_pc), ("wqt", aqk_pc)):
        for ci in range(CCH):
            if ci % 2 == 0:
                nc.vector.tensor_scalar_mul(wv_q[nm][:, ci, :], wv_q[nm][:, ci, :], avec[ci])
            else:
                nc.scalar.activation(out=wv_q[nm][:, ci, :], in_=wv_q[nm][:, ci, :],
                                     func=AF.Copy, scale=avec[ci])

    # ---------------- projections (fp8 DoubleRow) ----------------
    # K/Q: weight-stationary blocks -- within a pt-block the same lhsT serves
    # consecutive matmuls, so all but the first skip the (dominant, ~213ns)
    # DoubleRow LDWEIGHTS via ldweights=False. Order safety is asserted
    # post-build by _check_ldw_order. Evacuations split ACT/DVE.
    ksb = kpool.tile([128, CCH, HW], f8, tag="ksb", name="ksb")
    qsb = qpool.tile([128, CCH, HWQ], f8, tag="qsb", name="qsb")

    def proj_kq(wname, npt, dst, beff):
        evc = {"i": 0}
        for ptb in [range(b, min(b + 2, npt)) for b in range(0, npt, 2)]:
            for co in range(CCH):
                pss = {pt: ps_s.tile([128, 512], f32, tag="ps", name="ps")
                       for pt in ptb}
                for j in range(2):
                    for i, pt in enumerate(ptb):
                        mm = nc.tensor.matmul(
                            out=pss[pt],
                            lhsT=wv_q[wname][:, 2 * j:2 * j + 2, co * 128:(co + 1) * 128],
                            rhs=xkv_v[:, 2 * j:2 * j + 2, pt * 512:(pt + 1) * 512],
                            perf_mode=DR, start=(j == 0), stop=(j == 1))
                        if i > 0:
                            mm.ins.ldweights = False
                for pt in ptb:
                    if evc["i"] % 2 == 0:
                        nc.scalar.activation(out=dst[:, co, pt * 512:(pt + 1) * 512],
                                             in_=pss[pt], func=AF.Identity,
                                             bias=beff[co], scale=1.0)
                    else:
                        nc.vector.tensor_scalar_add(dst[:, co, pt * 512:(pt + 1) * 512],
                                                    pss[pt], beff[co])
                    evc["i"] += 1

    proj_kq("wkt", HW // 512, ksb, bkeff)
    proj_kq("wqt", HWQ // 512, qsb, bqeff)
    # V^T pair tiles [128, 2, 512] fp8: projected lazily inside the qt==0
    # attention loop (pair p lands just before its first AV consumer), so the
    # V matmuls fill PE slack and V evacuations ride the idle DVE while ACT
    # is exp-bound.
    vsb = [vpool.tile([128, 2, C], f8, tag="vt", name="vt") for _ in range(KT // 2)]

    def emit_vproj(kt):
        ps = ps_s.tile([128, 512], f32, tag="ps", name="ps")
        for j in range(2):
            nc.tensor.matmul(out=ps,
                             lhsT=xkv_v[:, 2 * j:2 * j + 2, kt * 128:(kt + 1) * 128],
                             rhs=wv_q["wvt"][:, 2 * j:2 * j + 2, :],
                             perf_mode=DR, start=(j == 0), stop=(j == 1))
        nc.vector.tensor_add(vsb[kt // 2][:, kt % 2, :], ps, bvb_sb)

    # ---------------- attention (+ deferred per-tile output projection) ----------------
    def emit_oproj_qc(qt, o_qt, rzc_sb, qc):
        ps = ps_s.tile([128, 512], f32, tag="ps", name="ps")
        for j in range(2):
            nc.tensor.matmul(out=ps, lhsT=o_qt[:, 2 * j:2 * j + 2, qc * 128:(qc + 1) * 128],
                             rhs=wot[:, 2 * j:2 * j + 2, :], perf_mode=DR,
                             start=(j == 0), stop=(j == 1))
        ot = outp.tile([128, 512], f16, tag="ot", name="ot")
        nc.vector.tensor_scalar_mul(ot, ps, rzc_sb[:, qc:qc + 1])
        nc.sync.dma_start(out=T["outt"][qt * 512 + qc * 128:qt * 512 + (qc + 1) * 128, :],
                          in_=ot)

    def emit_rz(ps_zt):
        """1/Z as [128, 4] (q on partitions): reciprocal row, then transpose
        each 128-slice via a trivial outer-product matmul (no DRAM trip)."""
        rz_row = rzp.tile([1, 512], f32, tag="rzrow", name="rzrow")
        nc.vector.reciprocal(out=rz_row, in_=ps_zt)
        ps_rz = ps_s.tile([128, 4], f32, tag="ps", name="ps")
        for qc in range(4):
            nc.tensor.matmul(out=ps_rz[:, qc:qc + 1],
                             lhsT=rz_row[0:1, qc * 128:(qc + 1) * 128],
                             rhs=ones1f[0:1, 0:1], start=True, stop=True)
        rzc_sb = rzp.tile([128, 4], f32, tag="rzc", name="rzc")
        nc.vector.tensor_copy(rzc_sb, ps_rz)
        return rzc_sb

    pending = None
    for qt in range(QT):
        ps_ot = [ps_o.tile([128, 512], f32, tag="pso", name="pso") for _ in range(CCH)]
        ps_zt = ps_z.tile([1, 512], f32, tag="z", name="z")
        prev_pair = None
        e_pair = None
        for kt in range(KT):
            pair, r = kt // 2, kt % 2
            if qt == 0:
                emit_vproj(kt)
            ps_st = ps_s.tile([128, 512], f32, tag="ps", name="ps")
            for j in range(2):
                nc.tensor.matmul(out=ps_st,
                                 lhsT=ksb[:, 2 * j:2 * j + 2, kt * 128:(kt + 1) * 128],
                                 rhs=qsb[:, 2 * j:2 * j + 2, qt * 512:(qt + 1) * 512],
                                 perf_mode=DR, start=(j == 0), stop=(j == 1))
            if r == 0:
                e_pair = epool.tile([128, 2, 512], f8, tag="e", name="e")
            nc.scalar.activation(out=e_pair[:, r, :], in_=ps_st, func=AF.Exp)
            if r == 0 and prev_pair is not None:
                ppair, pe = prev_pair
                nc.tensor.matmul(out=ps_zt, lhsT=ones2_sb[:, :, 0:1], rhs=pe, perf_mode=DR,
                                 start=(ppair == 0), stop=False, skip_group_check=True)
                for cc in range(CCH):
                    nc.tensor.matmul(out=ps_ot[cc],
                                     lhsT=vsb[ppair][:, :, cc * 128:(cc + 1) * 128],
                                     rhs=pe, perf_mode=DR, start=(ppair == 0),
                                     stop=False, skip_group_check=True)
            if r == 1:
                prev_pair = (pair, e_pair)
            # spread the previous qt's output projection across this qt's kt
            # loop (one qc every 4 kt) so its PSUM tiles and evacuations never
            # bunch up against the score pipeline
            if pending is not None and kt >= 5 and (kt - 5) % 4 == 0:
                emit_oproj_qc(*pending, qc=(kt - 5) // 4)
                if kt == 17:
                    pending = None
        ppair, pe = prev_pair
        nc.tensor.matmul(out=ps_zt, lhsT=ones2_sb[:, :, 0:1], rhs=pe, perf_mode=DR,
                         start=False, stop=True, skip_group_check=True)
        o_qt = opool.tile([128, CCH, 512], f8, tag=f"o{qt}", name=f"o{qt}")
        if qt < QT - 1:
            # evacuate each accumulator bank right after its closing matmul,
            # alternating DVE/ACT so the boundary chain isn't serial on DVE
            for cc in range(CCH):
                nc.tensor.matmul(out=ps_ot[cc],
                                 lhsT=vsb[ppair][:, :, cc * 128:(cc + 1) * 128],
                                 rhs=pe, perf_mode=DR, start=False, stop=True,
                                 skip_group_check=True)
                if cc % 2 == 0:
                    nc.vector.tensor_scalar_mul(o_qt[:, cc, :], ps_ot[cc], OSC)
                else:
                    nc.scalar.activation(out=o_qt[:, cc, :], in_=ps_ot[cc],
                                         func=AF.Copy, scale=OSC)
            rzc_sb = emit_rz(ps_zt)
            pending = (qt, o_qt, rzc_sb)
        else:
            for cc in range(CCH):
                nc.tensor.matmul(out=ps_ot[cc],
                                 lhsT=vsb[ppair][:, :, cc * 128:(cc + 1) * 128],
                                 rhs=pe, perf_mode=DR, start=False, stop=True,
                                 skip_group_check=True)
            rzc_sb = emit_rz(ps_zt)
            pending = (qt, o_qt, rzc_sb)
    # final tile: per-qc slice copies interleaved with its output projection,
    # evacuations alternating DVE/ACT (both idle by now)
    qt, o_qt, rzc_sb = pending
    for qc in range(4):
        for cc in range(CCH):
            if (qc + cc) % 2 == 0:
                nc.vector.tensor_scalar_mul(o_qt[:, cc, qc * 128:(qc + 1) * 128],
                                            ps_ot[cc][:, qc * 128:(qc + 1) * 128], OSC)
            else:
                nc.scalar.activation(out=o_qt[:, cc, qc * 128:(qc + 1) * 128],
                                     in_=ps_ot[cc][:, qc * 128:(qc + 1) * 128],
                                     func=AF.Copy, scale=OSC)
        ps = ps_s.tile([128, 512], f32, tag="ps", name="ps")
        for j in range(2):
            nc.tensor.matmul(out=ps, lhsT=o_qt[:, 2 * j:2 * j + 2, qc * 128:(qc + 1) * 128],
                             rhs=wot[:, 2 * j:2 * j + 2, :], perf_mode=DR,
                             start=(j == 0), stop=(j == 1))
        ot = outp.tile([128, 512], f16, tag="ot", name="ot")
        if qc % 2 == 0:
            nc.scalar.activation(out=ot, in_=ps, func=AF.Copy, scale=rzc_sb[:, qc:qc + 1])
        else:
            nc.vector.tensor_scalar_mul(ot, ps, rzc_sb[:, qc:qc + 1])
        nc.sync.dma_start(out=T["outt"][qt * 512 + qc * 128:qt * 512 + (qc + 1) * 128, :],
                          in_=ot)


def build_program(repeat=1):
    import concourse.bacc as bacc
    import concourse.tile as tile
    import concourse.bass as bass
    from concourse import mybir
    import contextlib

    f32 = mybir.dt.float32
    nc = bacc.Bacc(None, target_bir_lowering=False)

    # Every ACT func used here (Ln/Exp/Square/Identity/Copy) lives in the
    # natural_log_exp_and_others table set, but the table-load pass greedily
    # picks the first set containing each func, splitting across three sets
    # (3 loads, one mid-prefix). Empty the other sets in the cached table dict
    # (set ids are positional, so ids stay valid) -> exactly one table load.
    import concourse.hw_specs as hw_specs
    tabs = hw_specs.get_activation_tables(nc.m.arch)
    if "natural_log_exp_and_others" in tabs:
        keep = tabs["natural_log_exp_and_others"]
        need = {mybir.ActivationFunctionType.Ln, mybir.ActivationFunctionType.Exp,
                mybir.ActivationFunctionType.Square, mybir.ActivationFunctionType.Identity,
                mybir.ActivationFunctionType.Copy}
        if need <= keep:
            for name in tabs:
                if name != "natural_log_exp_and_others":
                    tabs[name].clear()

    T = {}
    f8 = mybir.dt.float8e4
    T["xkv"] = nc.dram_tensor("xkv", [C, HW], f8, kind="ExternalInput")
    T["wall"] = nc.dram_tensor("wall", [128, 12 * 512], f8, kind="ExternalInput")
    T["wotp"] = nc.dram_tensor("wotp", [128, 4 * 512], f8, kind="ExternalInput")
    T["cg"] = nc.dram_tensor("cg", [128, 144], f32, kind="ExternalInput")
    T["bvr"] = nc.dram_tensor("bvr", [C], f32, kind="ExternalInput")
    T["outt"] = nc.dram_tensor("outt", [HWQ, C], mybir.dt.float16, kind="ExternalOutput")

    with tile.TileContext(nc) as tc:
        for _ in range(repeat):
            with contextlib.ExitStack() as ctx:
                _emit_body(nc, tc, bass, mybir, ctx, T)
    nc.finalize()
    _strip_redundant_ldw(nc, mybir)
    return nc


def _strip_redundant_ldw(nc, mybir):
    """bacc lowers every matmul to InstLdweights + InstMatmult(ldweights=
    False). When consecutive PE-stream entries load the IDENTICAL stationary
    operand (the weight-stationary projection blocks above), the repeat
    Ldweights is a pure ~213ns reload of the same array state: drop it. Only
    drops sync-free Ldweights whose predecessor on the PE queue is an
    identical load (same AP/perf_mode/transpose/tile_position), so the
    matmuls' runtime weights are unchanged."""
    n_drop = 0
    for b in nc.main_func.blocks:
        keep = []
        prev_key = None
        for i in b.instructions:
            if isinstance(i, mybir.InstLdweights):
                key = (str(i.ins[0]), str(i.perf_mode), str(i.is_transpose),
                       str(i.tile_position))
                if (key == prev_key and not i.has_wait() and not i.has_update()):
                    n_drop += 1
                    continue
                prev_key = key
            elif isinstance(i, mybir.InstMatmult):
                pass       # matmuls between identical loads don't invalidate them
            elif getattr(i, "engine", None) == mybir.EngineType.PE:
                prev_key = None
            keep.append(i)
        b.instructions[:] = keep
    return n_drop


def make_in_maps(inputs):
    """Host-side sharding: per-core input dicts."""
    x = np.ascontiguousarray(np.asarray(inputs["x"], dtype=np.float32))
    B = x.shape[0]
    xf = x.reshape(B, C, HW)
    f8 = ml_dtypes.float8_e4m3
    wT8 = {nm: np.asarray(inputs[nm], np.float32).T.astype(f8)
           for nm in ("wq", "wk", "wv")}
    wall = np.empty((128, 12 * 512), f8)
    for i, nm in enumerate(("wk", "wv", "wq")):
        for ci in range(CCH):
            wall[:, i * 2048 + ci * 512:i * 2048 + (ci + 1) * 512] = \
                wT8[nm][ci * 128:(ci + 1) * 128, :]
    # 1/OSC folded into wot so the final scale is a single 1/Z multiply
    woT = (np.asarray(inputs["wo"], np.float32).T / OSC).astype(f8)
    wotp = np.empty((128, 4 * 512), f8)
    for ci in range(CCH):
        wotp[:, ci * 512:(ci + 1) * 512] = woT[ci * 128:(ci + 1) * 128, :]
    cg = np.empty((128, 144), np.float32)
    for ci in range(CCH):
        sl = slice(ci * 128, (ci + 1) * 128)
        cg[:, 4 * ci + 0] = np.asarray(inputs["gn_w"], np.float32)[sl]
        cg[:, 4 * ci + 1] = np.asarray(inputs["gn_b"], np.float32)[sl]
        cg[:, 4 * ci + 2] = np.asarray(inputs["bq"], np.float32)[sl] * SCALE_H
        cg[:, 4 * ci + 3] = np.asarray(inputs["bk"], np.float32)[sl] * SCALE_H
    cg[:, 16:144] = (np.arange(128)[:, None] // GS ==
                     np.arange(128)[None, :] // GS).astype(np.float32)
    com = {
        "wall": np.ascontiguousarray(wall),
        "wotp": np.ascontiguousarray(wotp),
        "cg": np.ascontiguousarray(cg),
        "bvr": np.ascontiguousarray(np.asarray(inputs["bv"], np.float32)),
    }
    in_maps = []
    for core in range(NCORES):
        b, half = core // 2, core % 2
        m = dict(com)
        # roll x so this core's query half is columns 0:HWQ (K/V/stats are
        # column-order invariant; assemble() maps rows back per core)
        off = half * HWQ
        xr = np.concatenate([xf[b][:, off:], xf[b][:, :off]], axis=1) if off else xf[b]
        m["xkv"] = np.ascontiguousarray(xr).astype(f8)
        in_maps.append(m)
    return in_maps


def assemble(inputs, results):
    x = np.asarray(inputs["x"], dtype=np.float32)
    B = x.shape[0]
    xf = x.reshape(B, C, HW)
    bo = np.asarray(inputs["bo"], np.float32)
    out = np.empty((B, C, HW), np.float32)
    for core in range(NCORES):
        b, half = core // 2, core % 2
        out[b][:, half * HWQ:(half + 1) * HWQ] = results[core]["outt"].T.astype(np.float32)
    out += bo[None, :, None]
    out += xf
    return out.reshape(x.shape)


def kernel(**inputs):
    from concourse.bass_utils import run_bass_kernel_spmd
    if "nc" not in _cache:
        _cache["nc"] = build_program(repeat=1)
    nc = _cache["nc"]
    in_maps = make_in_maps(inputs)
    res = run_bass_kernel_spmd(nc, in_maps, list(range(NCORES)))
    return assemble(inputs, res.results)


# revision 32
# speedup vs baseline: 1.0071x; 1.0071x over previous
"""AttnBlock (GroupNorm + single-head self-attention + residual) on 8 Trainium2 cores.

Sharding: core i handles batch b = i//2 and query-half h = i%2 (2048 of 4096
pixels). Each core computes full-batch groupnorm stats + K/V^T, its half of Q,
attention over all 4096 keys for its 2048 queries, and the output projection.
Host does the final bias + residual add and gathers.

v2 schedule notes (vs v1):
  - x is rolled per core so this core's query half is always columns 0:2048 of
    xkv; Q projects from a slice of xkv (no separate xq DMA).
  - x DMA is chunked (4x 512KB); groupnorm stats pipeline per chunk.
  - rstd = exp(-0.5*ln(var+eps)) so every ACT func (Square/Ln/Exp/Copy/
    Identity) lives in one table set -> single table load.
  - C**-0.25 folded into the q/k weight scaling; K/Q evacuations are pure
    bias-adds on ScalarE, V evacuations on VectorE (splits the PSUM-
    evacuation load across both engines; v1 put everything on VectorE).
  - 1/OSC folded into wot on the host; o8/ot evacuations run on VectorE
    (idle during attention), final-tile ot evacuations on ScalarE.
  - PE warm-up matmuls read stats scratch / weight tiles (dependency-gated)
    instead of dedicated memset tiles.

All matmuls fp8 DoubleRow (fp32 PSUM accumulation); softmax stats in fp32.
Layouts (no on-chip transpose anywhere):
  - Q, K as [c, pixel]   (projection natural layout)
  - V^T as [pixel, c]    (computed directly: lhsT = x blocks)
  - scores S^T[k, q]     (lhsT = K blocks, rhs = Q)
  - softmax sum over k (partition dim) via ones-vector matmul
  - attention out o[c, q](lhsT = V^T blocks, rhs = E^T)
  - o-proj out[q, c']    (lhsT = o blocks, rhs = wo^T) -> per-partition 1/Z scale
"""

import numpy as np
import ml_dtypes

C = 512
HW = 4096
HWQ = 2048
CCH = 4          # channel chunks of 128
KT = 32          # key tiles of 128
QT = 4           # query tiles of 512
NCORES = 8
GS = 16          # channels per group
EPS = 1e-5
SCALE = 1.0 / float(np.sqrt(C))
SCALE_H = float(SCALE ** 0.5)
OSC = 1.0 / 32.0   # o scaled into fp8 range; 1/OSC is folded into wot on host

_cache = {}


def _emit_body(nc, tc, bassmod, mybir, ctx, T):
    """Emit one full forward pass. T is the dict of dram tensor handles."""
    bass = bassmod
    f32 = mybir.dt.float32
    f8 = mybir.dt.float8e4
    f16 = mybir.dt.float16
    AF = mybir.ActivationFunctionType
    ALU = mybir.AluOpType
    DR = mybir.MatmulPerfMode.DoubleRow

    # ---------------- pools ----------------
    consts = ctx.enter_context(tc.tile_pool(name="consts", bufs=1))
    xb = ctx.enter_context(tc.tile_pool(name="xb", bufs=1))
    ps_s = ctx.enter_context(tc.tile_pool(name="ps_s", bufs=3, space="PSUM"))
    ps_o = ctx.enter_context(tc.tile_pool(name="ps_o", bufs=4, space="PSUM"))
    ps_z = ctx.enter_context(tc.tile_pool(name="ps_z", bufs=1, space="PSUM"))
    kpool = ctx.enter_context(tc.tile_pool(name="kpool", bufs=1))
    qpool = ctx.enter_context(tc.tile_pool(name="qpool", bufs=1))
    vpool = ctx.enter_context(tc.tile_pool(name="vpool", bufs=KT // 2))
    opool = ctx.enter_context(tc.tile_pool(name="opool", bufs=1))
    epool = ctx.enter_context(tc.tile_pool(name="epool", bufs=6))
    outp = ctx.enter_context(tc.tile_pool(name="outp", bufs=4))
    rzp = ctx.enter_context(tc.tile_pool(name="rzp", bufs=2))
    spool = ctx.enter_context(tc.tile_pool(name="spool", bufs=1))
    tmpp = ctx.enter_context(tc.tile_pool(name="tmpp", bufs=2))

    # ---------------- input DMAs ----------------
    # Descriptor generation is ~650ns per dma_start and serial on the sync
    # queue: x chunks go FIRST (stats gate on them), consts after.
    xkv_v = xb.tile([128, CCH, HW], f8, tag="xkv", name="xkv")
    for ci in range(CCH):
        nc.sync.dma_start(out=xkv_v[:, ci, :],
                          in_=bass.AP(T["xkv"], ci * 128 * HW, [[HW, 128], [1, HW]]))
    # colc (16 cols) + gadj (128 cols) merged into one [128, 144] input
    cg_sb = consts.tile([128, 144], f32, tag="cg", name="cg")
    nc.sync.dma_start(out=cg_sb, in_=T["cg"][:, :])
    cc_sb = cg_sb[:, 0:16]
    gadj_sb = cg_sb[:, 16:144]
    bv_row = consts.tile([1, C], f32, tag="bvrow", name="bvrow")
    nc.sync.dma_start(out=bv_row, in_=bass.AP(T["bvr"], 0, [[0, 1], [1, C]]))
    gnw_c = [cc_sb[:, 4 * ci + 0:4 * ci + 1] for ci in range(CCH)]
    gnb_c = [cc_sb[:, 4 * ci + 1:4 * ci + 2] for ci in range(CCH)]
    bq_c = [cc_sb[:, 4 * ci + 2:4 * ci + 3] for ci in range(CCH)]
    bk_c = [cc_sb[:, 4 * ci + 3:4 * ci + 4] for ci in range(CCH)]
    # fp8 weights for q/k/v: [128, nm, ci, co]; fp8 wot (pre-scaled by 1/OSC)
    wall = consts.tile([128, 3 * CCH * 512], f8, tag="wall", name="wall")
    nc.sync.dma_start(out=wall, in_=T["wall"][:, :])
    wv_q = {nm: wall[:, i * 2048:(i + 1) * 2048].rearrange("p (c w) -> p c w", c=CCH)
            for i, nm in enumerate(("wkt", "wvt", "wqt"))}
    wot = consts.tile([128, CCH, 512], f8, tag="wot", name="wot")
    nc.sync.dma_start(out=wot, in_=T["wotp"][:, :].rearrange("p (c w) -> p c w", c=CCH))
    ones2_sb = consts.tile([128, 2, 16], f8, tag="ones2", name="ones2")
    nc.vector.memset(ones2_sb, 1.0)
    ones1f = consts.tile([1, 128], f32, tag="ones1f", name="ones1f")
    nc.vector.memset(ones1f, 1.0)
    eps_sb = consts.tile([128, 1], f32, tag="eps", name="eps")
    nc.vector.memset(eps_sb, EPS)

    # PE warm-up: HAM needs ~3.4us of activity to unthrottle and re-throttles
    # after ~3.4us idle. Warm matmuls read the stats scratch of each chunk so
    # they spread across the stats prefix via data deps (no dedicated gates).
    ps_w = ps_z.tile([1, 512], f32, tag="z", name="z")
    _warm = {"first": True}

    def warm_round(rhs3, n=6, last=False):
        for i in range(n):
            nc.tensor.matmul(out=ps_w, lhsT=ones2_sb[:, :, 0:1], rhs=rhs3,
                             perf_mode=DR, start=_warm["first"],
                             stop=(last and i == n - 1), skip_group_check=True)
            _warm["first"] = False

    # ---------------- groupnorm stats (pipelined per chunk) ----------------
    # Per chunk: plain sum on VectorE (chunk 0: GpSimd/Pool, the only engine
    # otherwise idle in the prefix), sum-of-squares via ScalarE Square+accum.
    # The last chunk's stats gate the weight scaling, so spreading the passes
    # across three engines shortens the serial prefix.
    a_pc, aqk_pc = [], []
    b8 = spool.tile([128, CCH], f8, tag="b8", name="b8")
    var4 = spool.tile([128, CCH], f32, tag="var4", name="var4")
    mu4 = spool.tile([128, CCH], f32, tag="mu4", name="mu4")
    for ci in range(CCH):
        st = spool.tile([128, 2], f32, tag=f"st{ci}", name=f"st{ci}")
        nc.vector.reduce_sum(out=st[:, 0:1], in_=xkv_v[:, ci, :],
                             axis=mybir.AxisListType.X)
        scr = tmpp.tile([128, HW], f8, tag="scr", name="scr")
        nc.scalar.activation(out=scr, in_=xkv_v[:, ci, :], func=AF.Square,
                             accum_out=st[:, 1:2])
        ps_g = ps_s.tile([128, 2], f32, tag="ps", name="ps")
        nc.tensor.matmul(out=ps_g, lhsT=gadj_sb, rhs=st, start=True, stop=True)
        gs = spool.tile([128, 2], f32, tag=f"gs{ci}", name=f"gs{ci}")
        nc.scalar.mul(out=gs, in_=ps_g, mul=1.0 / (GS * HW))   # [mu | E[x^2]]
        nc.vector.tensor_copy(mu4[:, ci:ci + 1], gs[:, 0:1])
        var = var4[:, ci:ci + 1]
        nc.vector.tensor_mul(var, gs[:, 0:1], gs[:, 0:1])
        nc.vector.tensor_sub(var, gs[:, 1:2], var)
        warm_round(scr[:, 0:1024].rearrange("p (a b) -> p a b", a=2))
    # rstd = (var+eps)^-0.5 = exp(-0.5*ln(var+eps)), batched over chunks:
    # Ln/Exp live in the resident table set, so no mid-kernel table reloads.
    # All per-chunk [128,1] algebra is batched into [128,4] ops -- the serial
    # chain of tiny ops (+~100ns sem delay each) sat on the prefix critical
    # path between the last chunk's stats and the weight scaling.
    lnv = spool.tile([128, CCH], f32, tag="lnv", name="lnv")
    nc.scalar.activation(out=lnv, in_=var4, func=AF.Ln, bias=eps_sb, scale=1.0)
    rstd4 = spool.tile([128, CCH], f32, tag="rstd4", name="rstd4")
    nc.scalar.activation(out=rstd4, in_=lnv, func=AF.Exp, scale=-0.5)
    a4 = spool.tile([128, CCH], f32, tag="a4", name="a4")
    nc.vector.tensor_mul(a4, rstd4, cc_sb[:, 0:16:4])          # rstd * gn_w
    aqk4 = spool.tile([128, CCH], f32, tag="aqk4", name="aqk4")
    nc.vector.tensor_scalar_mul(aqk4, a4, SCALE_H)
    b4 = spool.tile([128, CCH], f32, tag="b4", name="b4")
    nc.vector.tensor_mul(b4, mu4, a4)
    nc.vector.tensor_sub(b4, cc_sb[:, 1:16:4], b4)             # gn_b - mu*a
    nc.vector.tensor_copy(b8, b4)
    for ci in range(CCH):
        a_pc.append(a4[:, ci:ci + 1])
        aqk_pc.append(aqk4[:, ci:ci + 1])

    # effective projection biases: beta = W @ b + bias  (original weights, fp8 matvecs)
    bqeff, bkeff = [], []
    for co in range(CCH):
        psq = ps_s.tile([128, 1], f32, tag="ps", name="ps")
        psk = ps_s.tile([128, 1], f32, tag="ps", name="ps")
        for ci in range(CCH):
            nc.tensor.matmul(out=psk, lhsT=wv_q["wkt"][:, ci, co * 128:(co + 1) * 128],
                             rhs=b8[:, ci:ci + 1], start=(ci == 0), stop=(ci == CCH - 1))
        for ci in range(CCH):
            nc.tensor.matmul(out=psq, lhsT=wv_q["wqt"][:, ci, co * 128:(co + 1) * 128],
                             rhs=b8[:, ci:ci + 1], start=(ci == 0), stop=(ci == CCH - 1))
        bk = spool.tile([128, 1], f32, tag=f"bke{co}", name=f"bke{co}")
        nc.scalar.activation(out=bk, in_=psk, func=AF.Identity, bias=bk_c[co], scale=SCALE_H)
        bq = spool.tile([128, 1], f32, tag=f"bqe{co}", name=f"bqe{co}")
        nc.scalar.activation(out=bq, in_=psq, func=AF.Identity, bias=bq_c[co], scale=SCALE_H)
        bqeff.append(bq)
        bkeff.append(bk)
    psv = ps_s.tile([1, C], f32, tag="ps", name="ps")
    for ci in range(CCH):
        nc.tensor.matmul(out=psv, lhsT=b8[:, ci:ci + 1], rhs=wv_q["wvt"][:, ci, :],
                         start=(ci == 0), stop=(ci == CCH - 1))
    bve_row = spool.tile([1, C], f32, tag="bverow", name="bverow")
    nc.vector.tensor_add(bve_row, psv, bv_row)
    # broadcast bve across partitions via outer-product matmul (no DRAM trip)
    ps_bvb = ps_s.tile([128, C], f32, tag="ps", name="ps")
    nc.tensor.matmul(out=ps_bvb, lhsT=ones1f, rhs=bve_row, start=True, stop=True)
    bvb_sb = consts.tile([128, C], f32, tag="bvb", name="bvb")
    nc.vector.tensor_copy(bvb_sb, ps_bvb)

    # scale q/k weight rows by a*C**-0.25 and v rows by a (in place, after the
    # beta matvecs read them); alternate DVE/ACT to split the chain
    warm_round(wv_q["wvt"][:, 0:2, 0:512], n=2, last=True)
    for nm, avec in (("wkt", aqk_pc), ("wvt", a_pc), ("wqt", aqk_pc)):
        for ci in range(CCH):
            if ci % 2 == 0:
                nc.vector.tensor_scalar_mul(wv_q[nm][:, ci, :], wv_q[nm][:, ci, :], avec[ci])
            else:
                nc.scalar.activation(out=wv_q[nm][:, ci, :], in_=wv_q[nm][:, ci, :],
                                     func=AF.Copy, scale=avec[ci])

    # ---------------- projections (fp8 DoubleRow) ----------------
    # K/Q: weight-stationary blocks -- within a pt-block the same lhsT serves
    # consecutive matmuls, so all but the first skip the (dominant, ~213ns)
    # DoubleRow LDWEIGHTS via ldweights=False. Order safety is asserted
    # post-build by _check_ldw_order. Evacuations split ACT/DVE.
    ksb = kpool.tile([128, CCH, HW], f8, tag="ksb", name="ksb")
    qsb = qpool.tile([128, CCH, HWQ], f8, tag="qsb", name="qsb")

    def proj_kq(wname, npt, dst, beff):
        evc = {"i": 0}
        for ptb in [range(b, min(b + 2, npt)) for b in range(0, npt, 2)]:
            for co in range(CCH):
                pss = {pt: ps_s.tile([128, 512], f32, tag="ps", name="ps")
                       for pt in ptb}
                for j in range(2):
                    for i, pt in enumerate(ptb):
                        mm = nc.tensor.matmul(
                            out=pss[pt],
                            lhsT=wv_q[wname][:, 2 * j:2 * j + 2, co * 128:(co + 1) * 128],
                            rhs=xkv_v[:, 2 * j:2 * j + 2, pt * 512:(pt + 1) * 512],
                            perf_mode=DR, start=(j == 0), stop=(j == 1))
                        if i > 0:
                            mm.ins.ldweights = False
                for pt in ptb:
                    if evc["i"] % 2 == 0:
                        nc.scalar.activation(out=dst[:, co, pt * 512:(pt + 1) * 512],
                                             in_=pss[pt], func=AF.Identity,
                                             bias=beff[co], scale=1.0)
                    else:
                        nc.vector.tensor_scalar_add(dst[:, co, pt * 512:(pt + 1) * 512],
                                                    pss[pt], beff[co])
                    evc["i"] += 1

    proj_kq("wkt", HW // 512, ksb, bkeff)
    proj_kq("wqt", HWQ // 512, qsb, bqeff)
    # V^T pair tiles [128, 2, 512] fp8: projected lazily inside the qt==0
    # attention loop (pair p lands just before its first AV consumer), so the
    # V matmuls fill PE slack and V evacuations ride the idle DVE while ACT
    # is exp-bound.
    vsb = [vpool.tile([128, 2, C], f8, tag="vt", name="vt") for _ in range(KT // 2)]

    def emit_vproj(kt):
        ps = ps_s.tile([128, 512], f32, tag="ps", name="ps")
        for j in range(2):
            nc.tensor.matmul(out=ps,
                             lhsT=xkv_v[:, 2 * j:2 * j + 2, kt * 128:(kt + 1) * 128],
                             rhs=wv_q["wvt"][:, 2 * j:2 * j + 2, :],
                             perf_mode=DR, start=(j == 0), stop=(j == 1))
        nc.vector.tensor_add(vsb[kt // 2][:, kt % 2, :], ps, bvb_sb)

    # ---------------- attention (+ deferred per-tile output projection) ----------------
    def emit_oproj_qc(qt, o_qt, rzc_sb, qc):
        ps = ps_s.tile([128, 512], f32, tag="ps", name="ps")
        for j in range(2):
            nc.tensor.matmul(out=ps, lhsT=o_qt[:, 2 * j:2 * j + 2, qc * 128:(qc + 1) * 128],
                             rhs=wot[:, 2 * j:2 * j + 2, :], perf_mode=DR,
                             start=(j == 0), stop=(j == 1))
        ot = outp.tile([128, 512], f16, tag="ot", name="ot")
        nc.vector.tensor_scalar_mul(ot, ps, rzc_sb[:, qc:qc + 1])
        nc.sync.dma_start(out=T["outt"][qt * 512 + qc * 128:qt * 512 + (qc + 1) * 128, :],
                          in_=ot)

    def emit_rz(ps_zt):
        """1/Z as [128, 4] (q on partitions): reciprocal row, then transpose
        each 128-slice via a trivial outer-product matmul (no DRAM trip)."""
        rz_row = rzp.tile([1, 512], f32, tag="rzrow", name="rzrow")
        nc.vector.reciprocal(out=rz_row, in_=ps_zt)
        ps_rz = ps_s.tile([128, 4], f32, tag="ps", name="ps")
        for qc in range(4):
            nc.tensor.matmul(out=ps_rz[:, qc:qc + 1],
                             lhsT=rz_row[0:1, qc * 128:(qc + 1) * 128],
                             rhs=ones1f[0:1, 0:1], start=True, stop=True)
        rzc_sb = rzp.tile([128, 4], f32, tag="rzc", name="rzc")
        nc.vector.tensor_copy(rzc_sb, ps_rz)
        return rzc_sb

    pending = None
    for qt in range(QT):
        ps_ot = [ps_o.tile([128, 512], f32, tag="pso", name="pso") for _ in range(CCH)]
        ps_zt = ps_z.tile([1, 512], f32, tag="z", name="z")
        prev_pair = None
        e_pair = None
        for kt in range(KT):
            pair, r = kt // 2, kt % 2
            if qt == 0:
                emit_vproj(kt)
            ps_st = ps_s.tile([128, 512], f32, tag="ps", name="ps")
            for j in range(2):
                nc.tensor.matmul(out=ps_st,
                                 lhsT=ksb[:, 2 * j:2 * j + 2, kt * 128:(kt + 1) * 128],
                                 rhs=qsb[:, 2 * j:2 * j + 2, qt * 512:(qt + 1) * 512],
                                 perf_mode=DR, start=(j == 0), stop=(j == 1))
            if r == 0:
                e_pair = epool.tile([128, 2, 512], f8, tag="e", name="e")
            nc.scalar.activation(out=e_pair[:, r, :], in_=ps_st, func=AF.Exp)
            if r == 0 and prev_pair is not None:
                ppair, pe = prev_pair
                nc.tensor.matmul(out=ps_zt, lhsT=ones2_sb[:, :, 0:1], rhs=pe, perf_mode=DR,
                                 start=(ppair == 0), stop=False, skip_group_check=True)
                for cc in range(CCH):
                    nc.tensor.matmul(out=ps_ot[cc],
                                     lhsT=vsb[ppair][:, :, cc * 128:(cc + 1) * 128],
                                     rhs=pe, perf_mode=DR, start=(ppair == 0),
                                     stop=False, skip_group_check=True)
            if r == 1:
                prev_pair = (pair, e_pair)
            # spread the previous qt's output projection across this qt's kt
            # loop (one qc every 4 kt) so its PSUM tiles and evacuations never
            # bunch up against the score pipeline
            if pending is not None and kt >= 5 and (kt - 5) % 4 == 0:
                emit_oproj_qc(*pending, qc=(kt - 5) // 4)
                if kt == 17:
                    pending = None
        ppair, pe = prev_pair
        nc.tensor.matmul(out=ps_zt, lhsT=ones2_sb[:, :, 0:1], rhs=pe, perf_mode=DR,
                         start=False, stop=True, skip_group_check=True)
        o_qt = opool.tile([128, CCH, 512], f8, tag=f"o{qt}", name=f"o{qt}")
        if qt < QT - 1:
            # evacuate each accumulator bank right after its closing matmul,
            # alternating DVE/ACT so the boundary chain isn't serial on DVE
            for cc in range(CCH):
                nc.tensor.matmul(out=ps_ot[cc],
                                 lhsT=vsb[ppair][:, :, cc * 128:(cc + 1) * 128],
                                 rhs=pe, perf_mode=DR, start=False, stop=True,
                                 skip_group_check=True)
                if cc % 2 == 0:
                    nc.vector.tensor_scalar_mul(o_qt[:, cc, :], ps_ot[cc], OSC)
                else:
                    nc.scalar.activation(out=o_qt[:, cc, :], in_=ps_ot[cc],
                                         func=AF.Copy, scale=OSC)
            rzc_sb = emit_rz(ps_zt)
            pending = (qt, o_qt, rzc_sb)
        else:
            for cc in range(CCH):
                nc.tensor.matmul(out=ps_ot[cc],
                                 lhsT=vsb[ppair][:, :, cc * 128:(cc + 1) * 128],
                                 rhs=pe, perf_mode=DR, start=False, stop=True,
                                 skip_group_check=True)
            rzc_sb = emit_rz(ps_zt)
            pending = (qt, o_qt, rzc_sb)
    # final tile: per-qc slice copies interleaved with its output projection,
    # evacuations alternating DVE/ACT (both idle by now)
    qt, o_qt, rzc_sb = pending
    for qc in range(4):
        for cc in range(CCH):
            if (qc + cc) % 2 == 0:
                nc.vector.tensor_scalar_mul(o_qt[:, cc, qc * 128:(qc + 1) * 128],
                                            ps_ot[cc][:, qc * 128:(qc + 1) * 128], OSC)
            else:
                nc.scalar.activation(out=o_qt[:, cc, qc * 128:(qc + 1) * 128],
                                     in_=ps_ot[cc][:, qc * 128:(qc + 1) * 128],
                                     func=AF.Copy, scale=OSC)
        ps = ps_s.tile([128, 512], f32, tag="ps", name="ps")
        for j in range(2):
            nc.tensor.matmul(out=ps, lhsT=o_qt[:, 2 * j:2 * j + 2, qc * 128:(qc + 1) * 128],
                             rhs=wot[:, 2 * j:2 * j + 2, :], perf_mode=DR,
                             start=(j == 0), stop=(j == 1))
        ot = outp.tile([128, 512], f16, tag="ot", name="ot")
        if qc % 2 == 0:
            nc.scalar.activation(out=ot, in_=ps, func=AF.Copy, scale=rzc_sb[:, qc:qc + 1])
        else:
            nc.vector.tensor_scalar_mul(ot, ps, rzc_sb[:, qc:qc + 1])
        nc.sync.dma_start(out=T["outt"][qt * 512 + qc * 128:qt * 512 + (qc + 1) * 128, :],
                          in_=ot)


def build_program(repeat=1):
    import concourse.bacc as bacc
    import concourse.tile as tile
    import concourse.bass as bass
    from concourse import mybir
    import contextlib

    f32 = mybir.dt.float32
    nc = bacc.Bacc(None, target_bir_lowering=False)

    # Every ACT func used here (Ln/Exp/Square/Identity/Copy) lives in the
    # natural_log_exp_and_others table set, but the table-load pass greedily
    # picks the first set containing each func, splitting across three sets
    # (3 loads, one mid-prefix). Empty the other sets in the cached table dict
    # (set ids are positional, so ids stay valid) -> exactly one table load.
    import concourse.hw_specs as hw_specs
    tabs = hw_specs.get_activation_tables(nc.m.arch)
    if "natural_log_exp_and_others" in tabs:
        keep = tabs["natural_log_exp_and_others"]
        need = {mybir.ActivationFunctionType.Ln, mybir.ActivationFunctionType.Exp,
                mybir.ActivationFunctionType.Square, mybir.ActivationFunctionType.Identity,
                mybir.ActivationFunctionType.Copy}
        if need <= keep:
            for name in tabs:
                if name != "natural_log_exp_and_others":
                    tabs[name].clear()

    T = {}
    f8 = mybir.dt.float8e4
    T["xkv"] = nc.dram_tensor("xkv", [C, HW], f8, kind="ExternalInput")
    T["wall"] = nc.dram_tensor("wall", [128, 12 * 512], f8, kind="ExternalInput")
    T["wotp"] = nc.dram_tensor("wotp", [128, 4 * 512], f8, kind="ExternalInput")
    T["cg"] = nc.dram_tensor("cg", [128, 144], f32, kind="ExternalInput")
    T["bvr"] = nc.dram_tensor("bvr", [C], f32, kind="ExternalInput")
    T["outt"] = nc.dram_tensor("outt", [HWQ, C], mybir.dt.float16, kind="ExternalOutput")

    with tile.TileContext(nc) as tc:
        for _ in range(repeat):
            with contextlib.ExitStack() as ctx:
                _emit_body(nc, tc, bass, mybir, ctx, T)
    nc.finalize()
    _strip_redundant_ldw(nc, mybir)
    return nc


def _strip_redundant_ldw(nc, mybir):
    """bacc lowers every matmul to InstLdweights + InstMatmult(ldweights=
    False). When consecutive PE-stream entries load the IDENTICAL stationary
    operand (the weight-stationary projection blocks above), the repeat
    Ldweights is a pure ~213ns reload of the same array state: drop it. Only
    drops sync-free Ldweights whose predecessor on the PE queue is an
    identical load (same AP/perf_mode/transpose/tile_position), so the
    matmuls' runtime weights are unchanged."""
    n_drop = 0
    for b in nc.main_func.blocks:
        keep = []
        prev_key = None
        for i in b.instructions:
            if isinstance(i, mybir.InstLdweights):
                key = (str(i.ins[0]), str(i.perf_mode), str(i.is_transpose),
                       str(i.tile_position))
                if (key == prev_key and not i.has_wait() and not i.has_update()):
                    n_drop += 1
                    continue
                prev_key = key
            elif isinstance(i, mybir.InstMatmult):
                pass       # matmuls between identical loads don't invalidate them
            elif getattr(i, "engine", None) == mybir.EngineType.PE:
                prev_key = None
            keep.append(i)
        b.instructions[:] = keep
    return n_drop


def make_in_maps(inputs):
    """Host-side sharding: per-core input dicts."""
    x = np.ascontiguousarray(np.asarray(inputs["x"], dtype=np.float32))
    B = x.shape[0]
    xf = x.reshape(B, C, HW)
    f8 = ml_dtypes.float8_e4m3
    wT8 = {nm: np.asarray(inputs[nm], np.float32).T.astype(f8)
           for nm in ("wq", "wk", "wv")}
    wall = np.empty((128, 12 * 512), f8)
    for i, nm in enumerate(("wk", "wv", "wq")):
        for ci in range(CCH):
            wall[:, i * 2048 + ci * 512:i * 2048 + (ci + 1) * 512] = \
                wT8[nm][ci * 128:(ci + 1) * 128, :]
    # 1/OSC folded into wot so the final scale is a single 1/Z multiply
    woT = (np.asarray(inputs["wo"], np.float32).T / OSC).astype(f8)
    wotp = np.empty((128, 4 * 512), f8)
    for ci in range(CCH):
        wotp[:, ci * 512:(ci + 1) * 512] = woT[ci * 128:(ci + 1) * 128, :]
    cg = np.empty((128, 144), np.float32)
    for ci in range(CCH):
        sl = slice(ci * 128, (ci + 1) * 128)
        cg[:, 4 * ci + 0] = np.asarray(inputs["gn_w"], np.float32)[sl]
        cg[:, 4 * ci + 1] = np.asarray(inputs["gn_b"], np.float32)[sl]
        cg[:, 4 * ci + 2] = np.asarray(inputs["bq"], np.float32)[sl] * SCALE_H
        cg[:, 4 * ci + 3] = np.asarray(inputs["bk"], np.float32)[sl] * SCALE_H
    cg[:, 16:144] = (np.arange(128)[:, None] // GS ==
                     np.arange(128)[None, :] // GS).astype(np.float32)
    com = {
        "wall": np.ascontiguousarray(wall),
        "wotp": np.ascontiguousarray(wotp),
        "cg": np.ascontiguousarray(cg),
        "bvr": np.ascontiguousarray(np.asarray(inputs["bv"], np.float32)),
    }
    in_maps = []
    for core in range(NCORES):
        b, half = core // 2, core % 2
        m = dict(com)
        # roll x so this core's query half is columns 0:HWQ (K/V/stats are
        # column-order invariant; assemble() maps rows back per core)
        off = half * HWQ
        xr = np.concatenate([xf[b][:, off:], xf[b][:, :off]], axis=1) if off else xf[b]
        m["xkv"] = np.ascontiguousarray(xr).astype(f8)
        in_maps.append(m)
    return in_maps


def assemble(inputs, results):
    x = np.asarray(inputs["x"], dtype=np.float32)
    B = x.shape[0]
    xf = x.reshape(B, C, HW)
    bo = np.asarray(inputs["bo"], np.float32)
    out = np.empty((B, C, HW), np.float32)
    for core in range(NCORES):
        b, half = core // 2, core % 2
        out[b][:, half * HWQ:(half + 1) * HWQ] = results[core]["outt"].T.astype(np.float32)
    out += bo[None, :, None]
    out += xf
    return out.reshape(x.shape)


def kernel(**inputs):
    from concourse.bass_utils import run_bass_kernel_spmd
    if "nc" not in _cache:
        _cache["nc"] = build_program(repeat=1)
    nc = _cache["nc"]
    in_maps = make_in_maps(inputs)
    res = run_bass_kernel_spmd(nc, in_maps, list(range(NCORES)))
    return assemble(inputs, res.results)
